# revision 28
# baseline (speedup 1.0000x reference)
"""MoE ConditionalFeedForward (SwiGLU, top-2 of 8 experts) on 8 TRN2 NeuronCores.

Strategy: expert-parallel. Core e owns expert e's weights (w1/w2/w3 slices).
The host routes tokens: for each expert, gather the UNIQUE tokens referencing
it (a token whose two slots pick the same expert is computed once, padded to
CAP), each core computes
    y = (silu(x @ w1[e].T) * (x @ w3[e].T)) @ w2[e].T
densely for its gathered tokens, and the host scatters rows back into the
[T, A, D] output (duplicated into both slots where needed).

Per-core kernel layout (all host-pretransposed so every DMA is contiguous):
  xt  [128, 8*CAP]    xt[p, k*CAP+j] = x_g[j, k*128+p]     (tokens, transposed)
  w13 [11, 128, 4096] pair j of i-tiles: [w1|w3] of it=2j then it=2j+1
  w2s [11, 128, 2048] w2s[j,p,:] = w2[e].T i-tile pair j
  yt  [128, 8*CAP]    bf16, yt[p, k*CAP+j] = y_g[j, k*128+p] (output, transposed)

Phase A (per i-tile it of 22): h1T/h3T [128(i), CAP] = sum_k wT @ x tiles in
PSUM, then hT = silu(h1)*h3 into SBUF. Phase B (transposed): yT[d-tile] [128,
CAP] accumulated over the 22 i-tiles in PSUM (8 banks, one per d-tile), with
the w2 128x128 tile stationary and hT moving.

DMA model (measured): the 16 HW descriptor queues are SHARED by all engine
rings and serve ~350 GB/s aggregate in rough enqueue order; descriptors are
one per partition row (~130ns each, nearly size-independent), so 4KB+ rows
are required to reach full rate. Priority = enqueue order, so the whole
weight stream rides ONE compute-free ring (sync) in exact consumption order
(x, w13 halves 0..21, w2 pairs in phase-B order); delivery (~2.9us/pair)
then always beats PE consumption (~3.8us/pair) and phase A runs PE-bound
end-to-end. scalar carries only the x trigger so silus never queue behind a
backpressure-blocked trigger. Warmup matmuls accumulate zeros into the first
h1 PSUM bank (releases the HAM clock-gate during the ~13us DMA lead-in
without a dedicated bank or DCE-guard output). Exec time beyond the matmul
stream is fixed overhead: ~5.8us program start (excluded from the metric),
~2.3us final-bank cast+trigger+descriptor chain, ~8.8us end-of-NEFF epilogue.
"""

import numpy as np
from contextlib import ExitStack

import concourse.bass as bass
import concourse.bacc as bacc
import concourse.mybir as mybir
import concourse.tile as tile
from concourse.bass_utils import run_bass_kernel_spmd

E, I, D = 8, 2816, 1024
N_CORES = 8
NI, ND = I // 128, D // 128  # 22, 8

# storage dtype for weights/activations on-device: "bfloat16" (half HBM
# traffic, full PE rate) or "float32" (matmuls run as float32r, 2 cyc/row)
DT_NAME = "bfloat16"

_PROG_CACHE: dict = {}


def _build_program(cap: int, dt_name: str):
    DT = mybir.dt.float32r if dt_name == "float32" else getattr(mybir.dt, dt_name)
    f32 = mybir.dt.float32
    bf16 = mybir.dt.bfloat16
    NP = NI // 2  # w13/w2 DMAs batched as i-tile pairs
    nc = bacc.Bacc("TRN2", target_bir_lowering=False, debug=False)
    xt = nc.dram_tensor("xt", [128, ND * cap], DT, kind="ExternalInput").ap()
    w13 = nc.dram_tensor("w13", [NP, 128, 4 * D], DT, kind="ExternalInput").ap()
    w2s = nc.dram_tensor("w2s", [NP, 128, 2 * D], DT, kind="ExternalInput").ap()
    yt = nc.dram_tensor("yt", [128, ND * cap], bf16, kind="ExternalOutput").ap()

    with tile.TileContext(nc) as tc, ExitStack() as ctx:
        warmp = ctx.enter_context(tc.tile_pool(name="warm", bufs=1))
        xp = ctx.enter_context(tc.tile_pool(name="x", bufs=1))
        # w13p bufs=4: streaming pool — the sync-ring trigger for pair j
        # blocks until pair j-4's matmuls retire, which still keeps
        # delivery ~10us ahead of consumption. Fewer tile buffers also
        # shorten the end-of-program per-buffer semaphore check chain
        # (~115ns each) that the exec-time metric includes.
        w13p = ctx.enter_context(tc.tile_pool(name="w13", bufs=4))
        hp = ctx.enter_context(tc.tile_pool(name="h", bufs=NI))
        silp = ctx.enter_context(tc.tile_pool(name="sil", bufs=2))
        w2p = ctx.enter_context(tc.tile_pool(name="w2", bufs=1))
        yp = ctx.enter_context(tc.tile_pool(name="y", bufs=1))

        # DMA schedule (v3): the 16 HW descriptor queues are SHARED by all
        # engine rings and service descriptors in rough enqueue order at
        # ~350 GB/s aggregate; concurrently-enqueued transfers dilute each
        # other (a transfer completes only when its last descriptor drains
        # behind everything enqueued alongside it). So priority = enqueue
        # order, and the whole weight stream goes on ONE ring (sync, which
        # has no compute duties) in exact consumption order: x_h1, w13
        # halves s=0..21, then w2 halves in phase-B it-order. Delivery
        # (~2.9us/pair) then always beats PE consumption (~3.8us/pair).
        # scalar carries only x_h0 so its silus are never stuck behind a
        # backpressure-blocked DMA trigger; gpsimd is unused.
        xsb = xp.tile([128, ND * cap], DT)
        w13ts = [
            w13p.tile([128, 4 * D], DT, tag="w13", name=f"w13_{j}")
            for j in range(NP)
        ]
        w2ts = [
            w2p.tile([128, 2 * D], DT, tag=f"w2_{j}", name=f"w2_{j}")
            for j in range(NP)
        ]

        def w13_half(eng, s):
            j, h = s // 2, s % 2
            eng.dma_start(
                w13ts[j][:, h * 2 * D : (h + 1) * 2 * D],
                w13[j][:, h * 2 * D : (h + 1) * 2 * D],
            )

        # Descriptors are one-per-partition-row (~130ns service each,
        # roughly size-independent), so larger rows = more bandwidth:
        # x goes as ONE transfer (4KB+ rows) and w2 as pair transfers
        # (4KB rows) rather than halves (2KB rows).
        nc.scalar.dma_start(xsb[:], xt[:])
        # half 0 split into its w1 and w3 column blocks: the first 8
        # matmuls need only x + the w1 block, so their DMA gate drops by
        # 128 descriptors (~0.8us); 2KB rows are fine for just this pair
        nc.sync.dma_start(w13ts[0][:, 0:D], w13[0][:, 0:D])
        nc.sync.dma_start(w13ts[0][:, D : 2 * D], w13[0][:, D : 2 * D])
        for s in range(1, 2 * NP):
            w13_half(nc.sync, s)
        for j in range(NP):
            nc.sync.dma_start(w2ts[j][:], w2s[j])

        # PE warmup: matmuls on a zeroed tile release the HAM clock-gate
        # and cover the window until x + the first w13 half land (~13us).
        # They accumulate zeros directly INTO the first h1 PSUM bank, so no
        # separate warm bank (all 8 banks stay available: 6 hps + 2 free
        # for phase B's first banks) and no DCE-guard output is needed.
        hts = []
        with tc.tile_pool(name="hps", bufs=3, space="PSUM") as hps:
            wtile = warmp.tile([128, 640], DT)
            nc.vector.memset(wtile[:], 0.0)
            h1_first = hps.tile([128, cap], f32, tag="h1", name="h1")
            n_warm = 26
            for i in range(n_warm):
                nc.tensor.matmul(
                    h1_first[:],
                    wtile[:, 0:128],
                    wtile[:, 128 : 128 + cap],
                    start=(i == 0),
                    stop=False,
                )

            for j in range(NP):
                wt = w13ts[j]
                for half in range(2):
                    base = half * 2 * D
                    first = j == 0 and half == 0
                    if first:
                        h1 = h1_first
                    else:
                        h1 = hps.tile([128, cap], f32, tag="h1", name="h1")
                    h3 = hps.tile([128, cap], f32, tag="h3", name="h3")
                    for k in range(ND):
                        nc.tensor.matmul(
                            h1[:],
                            wt[:, base + k * 128 : base + (k + 1) * 128],
                            xsb[:, k * cap : (k + 1) * cap],
                            start=(k == 0 and not first),
                            stop=(k == ND - 1),
                        )
                    for k in range(ND):
                        nc.tensor.matmul(
                            h3[:],
                            wt[:, base + D + k * 128 : base + D + (k + 1) * 128],
                            xsb[:, k * cap : (k + 1) * cap],
                            start=(k == 0),
                            stop=(k == ND - 1),
                        )
                    sil = silp.tile([128, cap], f32)
                    nc.scalar.activation(
                        sil[:], h1[:], mybir.ActivationFunctionType.Silu
                    )
                    ht = hp.tile([128, cap], DT)
                    nc.vector.tensor_mul(ht[:], sil[:], h3[:])
                    hts.append(ht)

        # Phase B: yT[d-tile][128, cap] += w2tile.T @ hT (w2 stationary).
        # it-major prefix consumes w2 i-tiles in streaming-arrival order
        # (w2 is still arriving when B starts); k-major tail staggers bank
        # completion so PSUM drains overlap the remaining matmuls.
        # ITSPLIT=11 staggers bank completions every NI-ITSPLIT i-tiles
        # (~1.24us) — just above the ~1.04us descriptor-service time of one
        # 67KB drain, so the drain stream keeps pace with bank completion
        # and the last drain starts right after the last matmul.
        ITSPLIT = 11
        with tc.tile_pool(name="yps", bufs=1, space="PSUM") as yps:
            # allocate banks in REVERSE k order: the allocator hands the
            # most-recently-freed PSUM bank (freed by the last mult) to the
            # first allocation, so give that one to ypt[7] — written 8
            # matmuls after ypt[0] — instead of stalling B's first matmul
            ypt = [
                yps.tile([128, cap], f32, tag=f"yps_{k}", name=f"yps_{k}")
                for k in reversed(range(ND))
            ][::-1]
            ysb = yp.tile([128, ND * cap], bf16)
            for it in range(ITSPLIT):
                for k in range(ND):
                    nc.tensor.matmul(
                        ypt[k][:],
                        w2ts[it // 2][
                            :, (it % 2) * D + k * 128 : (it % 2) * D + (k + 1) * 128
                        ],
                        hts[it][:],
                        start=(it == 0),
                        stop=False,
                    )
            for k in range(ND):
                for it in range(ITSPLIT, NI):
                    nc.tensor.matmul(
                        ypt[k][:],
                        w2ts[it // 2][
                            :, (it % 2) * D + k * 128 : (it % 2) * D + (k + 1) * 128
                        ],
                        hts[it][:],
                        start=False,
                        stop=(it == NI - 1),
                    )
                dst = ysb[:, k * cap : (k + 1) * cap]
                nc.vector.tensor_copy(dst, ypt[k][:])
                if k < ND - 2:
                    # one transfer per bank (128 row-descriptors); gpsimd's
                    # software-DMA path is slow, keep drains on sync/scalar
                    eng = nc.sync if k % 2 == 0 else nc.scalar
                    eng.dma_start(yt[:, k * cap : (k + 1) * cap], dst)
                else:
                    # last two banks: split along PARTITIONS (same total
                    # descriptor count) so the two ~600ns triggers run on
                    # both rings in parallel and each queue serves only 4
                    # rows — halves the post-matmul drain latency
                    cols = slice(k * cap, (k + 1) * cap)
                    nc.sync.dma_start(yt[0:64, cols], dst[0:64, :])
                    nc.scalar.dma_start(yt[64:128, cols], dst[64:128, :])

    nc.compile()
    return nc


def _get_program(cap: int, dt_name: str):
    key = (cap, dt_name)
    if key not in _PROG_CACHE:
        _PROG_CACHE[key] = _build_program(cap, dt_name)
    return _PROG_CACHE[key]


def _np_dt(dt_name: str):
    if dt_name == "float32":
        return np.float32
    import ml_dtypes

    return ml_dtypes.bfloat16


def _prep_weights(w1, w3, w2, dt_name):
    """Per-expert pretransposed/tiled weight arrays (see module docstring)."""
    npdt = _np_dt(dt_name)
    w13_all, w2s_all = [], []
    for e in range(E):
        # [I, D] -> [it, c, k, p] -> [it, p, k, c] -> [it, 128, 1024]
        a1 = w1[e].reshape(NI, 128, ND, 128).transpose(0, 3, 2, 1).reshape(NI, 128, D)
        a3 = w3[e].reshape(NI, 128, ND, 128).transpose(0, 3, 2, 1).reshape(NI, 128, D)
        # pairs of i-tiles: [11, 128, 4096] = [w1|w3] for it=2j then it=2j+1
        a13 = np.concatenate([a1, a3], axis=2).reshape(NI // 2, 2, 128, 2 * D)
        w13_all.append(
            np.ascontiguousarray(a13.transpose(0, 2, 1, 3)).reshape(
                NI // 2, 128, 4 * D
            ).astype(npdt)
        )
        # w2[e] [D, I] -> T [I, D] -> [22, 128, 1024] -> pairs [11, 128, 2048]
        a2 = w2[e].T.reshape(NI // 2, 2, 128, D)
        w2s_all.append(
            np.ascontiguousarray(a2.transpose(0, 2, 1, 3)).reshape(
                NI // 2, 128, 2 * D
            ).astype(npdt)
        )
    return w13_all, w2s_all


def kernel(x, w1, w2, w3, expert_indices, _trace=False):
    x = np.asarray(x, dtype=np.float32)
    w1 = np.asarray(w1, dtype=np.float32)
    w2 = np.asarray(w2, dtype=np.float32)
    w3 = np.asarray(w3, dtype=np.float32)
    idx = np.asarray(expert_indices).astype(np.int64)
    T, A = idx.shape
    npdt = _np_dt(DT_NAME)

    # Dedup: a token whose two slots pick the SAME expert is computed once
    # on that expert's core and its row written to both output slots.
    tok_lists = [np.nonzero((idx == e).any(axis=1))[0] for e in range(E)]
    counts = np.array([len(t) for t in tok_lists], dtype=np.int64)

    w13_all, w2s_all = _prep_weights(w1, w3, w2, DT_NAME)

    out = np.empty((T * A, D), dtype=np.float32)
    remaining = counts.copy()
    done = np.zeros(E, dtype=np.int64)
    last_res = None
    while remaining.max() > 0:
        cap = min(512, max(32, int(-(-remaining.max() // 8)) * 8))
        nc = _get_program(cap, DT_NAME)
        in_maps = []
        core_tok = []  # per-core token ids handled this round
        for e in range(E):
            n = int(min(remaining[e], cap))
            toks = tok_lists[e][done[e] : done[e] + n]
            core_tok.append(toks)
            xg = np.zeros((cap, D), dtype=np.float32)
            xg[:n] = x[toks]
            # [cap, D] -> T [D, cap] -> [k, 128, cap] -> [128, k, cap]
            xt_host = np.ascontiguousarray(
                xg.T.reshape(ND, 128, cap).transpose(1, 0, 2)
            ).reshape(128, ND * cap).astype(npdt)
            in_maps.append({"xt": xt_host, "w13": w13_all[e], "w2s": w2s_all[e]})
            remaining[e] -= n
            done[e] += n
        last_res = run_bass_kernel_spmd(
            nc, in_maps, core_ids=list(range(N_CORES)), trace=_trace
        )
        for e in range(E):
            toks = core_tok[e]
            if len(toks):
                # yt [128, 8*cap] -> [p, k, j] -> y[j, k*128+p]
                ye = (
                    last_res.results[e]["yt"]
                    .astype(np.float32)
                    .reshape(128, ND, cap)
                    .transpose(2, 1, 0)
                    .reshape(cap, D)
                )
                rr, aa = np.nonzero(idx[toks] == e)  # rows/slots to scatter
                out[toks[rr] * A + aa] = ye[rr]

    result = out.reshape(T, A, D)
    if _trace:
        return result, last_res
    return result



# revision 29
# speedup vs baseline: 1.1289x; 1.1289x over previous
"""MoE ConditionalFeedForward (SwiGLU, top-2 of 8 experts) on 8 TRN2 NeuronCores.

Strategy: expert-parallel. Core e owns expert e's weights (w1/w2/w3 slices).
The host routes tokens: for each expert, gather the UNIQUE tokens referencing
it (a token whose two slots pick the same expert is computed once, padded to
CAP), each core computes
    y = (silu(x @ w1[e].T) * (x @ w3[e].T)) @ w2[e].T
densely for its gathered tokens, and the host scatters rows back into the
[T, A, D] output (duplicated into both slots where needed).

Per-core kernel layout (all host-pretransposed so every DMA is contiguous):
  xt  [128, 8*CAP]    xt[p, k*CAP+j] = x_g[j, k*128+p]     (tokens, transposed)
  w13 [11, 128, 4096] pair j of i-tiles: [w1|w3] of it=2j then it=2j+1
  w2s [11, 128, 2048] w2s[j,p,:] = w2[e].T i-tile pair j
  yt  [128, 8*CAP]    bf16, yt[p, k*CAP+j] = y_g[j, k*128+p] (output, transposed)

Phase A (per i-tile it of 22): h1T/h3T [128(i), CAP] = sum_k wT @ x tiles in
PSUM, then hT = silu(h1)*h3 into SBUF. Phase B (transposed): yT[d-tile] [128,
CAP] accumulated over the 22 i-tiles in PSUM (8 banks, one per d-tile), with
the w2 128x128 tile stationary and hT moving.

DMA model (measured): the 16 HW descriptor queues are SHARED by all engine
rings and serve ~350 GB/s aggregate in rough enqueue order; descriptors are
one per partition row (~130ns each, nearly size-independent), so 4KB+ rows
are required to reach full rate. Priority = enqueue order, so the whole
weight stream rides ONE compute-free ring (sync) in exact consumption order
(x, w13 halves 0..21, w2 pairs in phase-B order); delivery (~2.9us/pair)
then always beats PE consumption (~3.8us/pair) and phase A runs PE-bound
end-to-end. scalar carries only the x trigger so silus never queue behind a
backpressure-blocked trigger. Warmup matmuls accumulate zeros into the first
h1 PSUM bank (releases the HAM clock-gate during the ~13us DMA lead-in
without a dedicated bank or DCE-guard output). Exec time beyond the matmul
stream is fixed overhead: ~5.8us program start (excluded from the metric),
~2.3us final-bank cast+trigger+descriptor chain, ~8.8us end-of-NEFF epilogue.
"""

import numpy as np
from contextlib import ExitStack

import concourse.bass as bass
import concourse.bacc as bacc
import concourse.mybir as mybir
import concourse.tile as tile
from concourse.bass_utils import run_bass_kernel_spmd

E, I, D = 8, 2816, 1024
N_CORES = 8
NI, ND = I // 128, D // 128  # 22, 8

# storage dtype for weights/activations on-device: "bfloat16" (half HBM
# traffic, full PE rate) or "float32" (matmuls run as float32r, 2 cyc/row)
DT_NAME = "bfloat16"

_PROG_CACHE: dict = {}


def _build_program(cap: int, dt_name: str):
    DT = mybir.dt.float32r if dt_name == "float32" else getattr(mybir.dt, dt_name)
    f32 = mybir.dt.float32
    bf16 = mybir.dt.bfloat16
    NP = NI // 2  # w13/w2 DMAs batched as i-tile pairs
    nc = bacc.Bacc("TRN2", target_bir_lowering=False, debug=False)
    xt = nc.dram_tensor("xt", [128, ND * cap], DT, kind="ExternalInput").ap()
    w13 = nc.dram_tensor("w13", [NP, 128, 4 * D], DT, kind="ExternalInput").ap()
    w2s = nc.dram_tensor("w2s", [NP, 128, 2 * D], DT, kind="ExternalInput").ap()
    yt = nc.dram_tensor("yt", [128, ND * cap], bf16, kind="ExternalOutput").ap()

    with tile.TileContext(nc) as tc, ExitStack() as ctx:
        warmp = ctx.enter_context(tc.tile_pool(name="warm", bufs=1))
        xp = ctx.enter_context(tc.tile_pool(name="x", bufs=1))
        # w13p bufs=4: streaming pool — the sync-ring trigger for pair j
        # blocks until pair j-4's matmuls retire, which still keeps
        # delivery ~10us ahead of consumption. Fewer tile buffers also
        # shorten the end-of-program per-buffer semaphore check chain
        # (~115ns each) that the exec-time metric includes.
        w13p = ctx.enter_context(tc.tile_pool(name="w13", bufs=4))
        hp = ctx.enter_context(tc.tile_pool(name="h", bufs=NI))
        silp = ctx.enter_context(tc.tile_pool(name="sil", bufs=2))
        w2p = ctx.enter_context(tc.tile_pool(name="w2", bufs=1))
        yp = ctx.enter_context(tc.tile_pool(name="y", bufs=1))

        # DMA schedule (v3): the 16 HW descriptor queues are SHARED by all
        # engine rings and service descriptors in rough enqueue order at
        # ~350 GB/s aggregate; concurrently-enqueued transfers dilute each
        # other (a transfer completes only when its last descriptor drains
        # behind everything enqueued alongside it). So priority = enqueue
        # order, and the whole weight stream goes on ONE ring (sync, which
        # has no compute duties) in exact consumption order: x_h1, w13
        # halves s=0..21, then w2 halves in phase-B it-order. Delivery
        # (~2.9us/pair) then always beats PE consumption (~3.8us/pair).
        # scalar carries only x_h0 so its silus are never stuck behind a
        # backpressure-blocked DMA trigger; gpsimd is unused.
        xsb = xp.tile([128, ND * cap], DT)
        w13ts = [
            w13p.tile([128, 4 * D], DT, tag="w13", name=f"w13_{j}")
            for j in range(NP)
        ]
        w2ts = [
            w2p.tile([128, 2 * D], DT, tag=f"w2_{j}", name=f"w2_{j}")
            for j in range(NP)
        ]

        def w13_half(eng, s):
            j, h = s // 2, s % 2
            eng.dma_start(
                w13ts[j][:, h * 2 * D : (h + 1) * 2 * D],
                w13[j][:, h * 2 * D : (h + 1) * 2 * D],
            )

        # Descriptors are one-per-partition-row (~130ns service each,
        # roughly size-independent), so larger rows = more bandwidth:
        # x goes as ONE transfer (4KB+ rows) and w2 as pair transfers
        # (4KB rows) rather than halves (2KB rows).
        nc.scalar.dma_start(xsb[:], xt[:])
        # half 0 split into its w1 and w3 column blocks: the first 8
        # matmuls need only x + the w1 block, so their DMA gate drops by
        # 128 descriptors (~0.8us); 2KB rows are fine for just this pair
        nc.sync.dma_start(w13ts[0][:, 0:D], w13[0][:, 0:D])
        nc.sync.dma_start(w13ts[0][:, D : 2 * D], w13[0][:, D : 2 * D])
        for s in range(1, 2 * NP):
            w13_half(nc.sync, s)
        for j in range(NP):
            nc.sync.dma_start(w2ts[j][:], w2s[j])

        # PE warmup: matmuls on a zeroed tile release the HAM clock-gate
        # and cover the window until x + the first w13 half land (~13us).
        # They accumulate zeros directly INTO the first h1 PSUM bank, so no
        # separate warm bank (all 8 banks stay available: 6 hps + 2 free
        # for phase B's first banks) and no DCE-guard output is needed.
        hts = []
        with tc.tile_pool(name="hps", bufs=3, space="PSUM") as hps:
            wtile = warmp.tile([128, 640], DT)
            nc.vector.memset(wtile[:], 0.0)
            h1_first = hps.tile([128, cap], f32, tag="h1", name="h1")
            n_warm = 26
            for i in range(n_warm):
                nc.tensor.matmul(
                    h1_first[:],
                    wtile[:, 0:128],
                    wtile[:, 128 : 128 + cap],
                    start=(i == 0),
                    stop=False,
                )

            for j in range(NP):
                wt = w13ts[j]
                for half in range(2):
                    base = half * 2 * D
                    first = j == 0 and half == 0
                    if first:
                        h1 = h1_first
                    else:
                        h1 = hps.tile([128, cap], f32, tag="h1", name="h1")
                    h3 = hps.tile([128, cap], f32, tag="h3", name="h3")
                    for k in range(ND):
                        nc.tensor.matmul(
                            h1[:],
                            wt[:, base + k * 128 : base + (k + 1) * 128],
                            xsb[:, k * cap : (k + 1) * cap],
                            start=(k == 0 and not first),
                            stop=(k == ND - 1),
                        )
                    for k in range(ND):
                        nc.tensor.matmul(
                            h3[:],
                            wt[:, base + D + k * 128 : base + D + (k + 1) * 128],
                            xsb[:, k * cap : (k + 1) * cap],
                            start=(k == 0),
                            stop=(k == ND - 1),
                        )
                    sil = silp.tile([128, cap], f32)
                    nc.scalar.activation(
                        sil[:], h1[:], mybir.ActivationFunctionType.Silu
                    )
                    ht = hp.tile([128, cap], DT)
                    nc.vector.tensor_mul(ht[:], sil[:], h3[:])
                    hts.append(ht)

        # Phase B: yT[d-tile][128, cap] += w2tile.T @ hT (w2 stationary).
        # it-major prefix consumes w2 i-tiles in streaming-arrival order
        # (w2 is still arriving when B starts); k-major tail staggers bank
        # completion so PSUM drains overlap the remaining matmuls.
        # ITSPLIT=11 staggers bank completions every NI-ITSPLIT i-tiles
        # (~1.24us) — just above the ~1.04us descriptor-service time of one
        # 67KB drain, so the drain stream keeps pace with bank completion
        # and the last drain starts right after the last matmul.
        ITSPLIT = 11
        with tc.tile_pool(name="yps", bufs=1, space="PSUM") as yps:
            ypt = [
                yps.tile([128, cap], f32, tag=f"yps_{k}", name=f"yps_{k}")
                for k in range(ND)
            ]
            ysb = yp.tile([128, ND * cap], bf16)
            for it in range(ITSPLIT):
                for k in range(ND):
                    nc.tensor.matmul(
                        ypt[k][:],
                        w2ts[it // 2][
                            :, (it % 2) * D + k * 128 : (it % 2) * D + (k + 1) * 128
                        ],
                        hts[it][:],
                        start=(it == 0),
                        stop=False,
                    )
            for k in range(ND):
                for it in range(ITSPLIT, NI):
                    nc.tensor.matmul(
                        ypt[k][:],
                        w2ts[it // 2][
                            :, (it % 2) * D + k * 128 : (it % 2) * D + (k + 1) * 128
                        ],
                        hts[it][:],
                        start=False,
                        stop=(it == NI - 1),
                    )
                dst = ysb[:, k * cap : (k + 1) * cap]
                nc.vector.tensor_copy(dst, ypt[k][:])
                if k < ND - 2:
                    # one transfer per bank (128 row-descriptors); gpsimd's
                    # software-DMA path is slow, keep drains on sync/scalar
                    eng = nc.sync if k % 2 == 0 else nc.scalar
                    eng.dma_start(yt[:, k * cap : (k + 1) * cap], dst)
                else:
                    # last two banks: split along PARTITIONS (same total
                    # descriptor count) so the two ~600ns triggers run on
                    # both rings in parallel and each queue serves only 4
                    # rows — halves the post-matmul drain latency
                    cols = slice(k * cap, (k + 1) * cap)
                    nc.sync.dma_start(yt[0:64, cols], dst[0:64, :])
                    nc.scalar.dma_start(yt[64:128, cols], dst[64:128, :])

    nc.compile()
    return nc


def _get_program(cap: int, dt_name: str):
    key = (cap, dt_name)
    if key not in _PROG_CACHE:
        _PROG_CACHE[key] = _build_program(cap, dt_name)
    return _PROG_CACHE[key]


def _np_dt(dt_name: str):
    if dt_name == "float32":
        return np.float32
    import ml_dtypes

    return ml_dtypes.bfloat16


def _prep_weights(w1, w3, w2, dt_name):
    """Per-expert pretransposed/tiled weight arrays (see module docstring)."""
    npdt = _np_dt(dt_name)
    w13_all, w2s_all = [], []
    for e in range(E):
        # [I, D] -> [it, c, k, p] -> [it, p, k, c] -> [it, 128, 1024]
        a1 = w1[e].reshape(NI, 128, ND, 128).transpose(0, 3, 2, 1).reshape(NI, 128, D)
        a3 = w3[e].reshape(NI, 128, ND, 128).transpose(0, 3, 2, 1).reshape(NI, 128, D)
        # pairs of i-tiles: [11, 128, 4096] = [w1|w3] for it=2j then it=2j+1
        a13 = np.concatenate([a1, a3], axis=2).reshape(NI // 2, 2, 128, 2 * D)
        w13_all.append(
            np.ascontiguousarray(a13.transpose(0, 2, 1, 3)).reshape(
                NI // 2, 128, 4 * D
            ).astype(npdt)
        )
        # w2[e] [D, I] -> T [I, D] -> [22, 128, 1024] -> pairs [11, 128, 2048]
        a2 = w2[e].T.reshape(NI // 2, 2, 128, D)
        w2s_all.append(
            np.ascontiguousarray(a2.transpose(0, 2, 1, 3)).reshape(
                NI // 2, 128, 2 * D
            ).astype(npdt)
        )
    return w13_all, w2s_all


def kernel(x, w1, w2, w3, expert_indices, _trace=False):
    x = np.asarray(x, dtype=np.float32)
    w1 = np.asarray(w1, dtype=np.float32)
    w2 = np.asarray(w2, dtype=np.float32)
    w3 = np.asarray(w3, dtype=np.float32)
    idx = np.asarray(expert_indices).astype(np.int64)
    T, A = idx.shape
    npdt = _np_dt(DT_NAME)

    # Dedup: a token whose two slots pick the SAME expert is computed once
    # on that expert's core and its row written to both output slots.
    tok_lists = [np.nonzero((idx == e).any(axis=1))[0] for e in range(E)]
    counts = np.array([len(t) for t in tok_lists], dtype=np.int64)

    w13_all, w2s_all = _prep_weights(w1, w3, w2, DT_NAME)

    out = np.empty((T * A, D), dtype=np.float32)
    remaining = counts.copy()
    done = np.zeros(E, dtype=np.int64)
    last_res = None
    while remaining.max() > 0:
        cap = min(512, max(32, int(-(-remaining.max() // 8)) * 8))
        nc = _get_program(cap, DT_NAME)
        in_maps = []
        core_tok = []  # per-core token ids handled this round
        for e in range(E):
            n = int(min(remaining[e], cap))
            toks = tok_lists[e][done[e] : done[e] + n]
            core_tok.append(toks)
            xg = np.zeros((cap, D), dtype=np.float32)
            xg[:n] = x[toks]
            # [cap, D] -> T [D, cap] -> [k, 128, cap] -> [128, k, cap]
            xt_host = np.ascontiguousarray(
                xg.T.reshape(ND, 128, cap).transpose(1, 0, 2)
            ).reshape(128, ND * cap).astype(npdt)
            in_maps.append({"xt": xt_host, "w13": w13_all[e], "w2s": w2s_all[e]})
            remaining[e] -= n
            done[e] += n
        last_res = run_bass_kernel_spmd(
            nc, in_maps, core_ids=list(range(N_CORES)), trace=_trace
        )
        for e in range(E):
            toks = core_tok[e]
            if len(toks):
                # yt [128, 8*cap] -> [p, k, j] -> y[j, k*128+p]
                ye = (
                    last_res.results[e]["yt"]
                    .astype(np.float32)
                    .reshape(128, ND, cap)
                    .transpose(2, 1, 0)
                    .reshape(cap, D)
                )
                rr, aa = np.nonzero(idx[toks] == e)  # rows/slots to scatter
                out[toks[rr] * A + aa] = ye[rr]

    result = out.reshape(T, A, D)
    if _trace:
        return result, last_res
    return result



# revision 30
# speedup vs baseline: 1.1863x; 1.0508x over previous
"""MoE ConditionalFeedForward (SwiGLU, top-2 of 8 experts) on 8 TRN2 NeuronCores.

Strategy: expert-parallel. Core e owns expert e's weights (w1/w2/w3 slices).
The host routes tokens: for each expert, gather the UNIQUE tokens referencing
it (a token whose two slots pick the same expert is computed once, padded to
CAP), each core computes
    y = (silu(x @ w1[e].T) * (x @ w3[e].T)) @ w2[e].T
densely for its gathered tokens, and the host scatters rows back into the
[T, A, D] output (duplicated into both slots where needed).

Per-core kernel layout (all host-pretransposed so every DMA is contiguous):
  xt  [128, 8*CAP]    xt[p, k*CAP+j] = x_g[j, k*128+p]     (tokens, transposed)
  w13 [11, 128, 4096] pair j of i-tiles: [w1|w3] of it=2j then it=2j+1
  w2s [11, 128, 2048] w2s[j,p,:] = w2[e].T i-tile pair j
  yt  [128, 8*CAP]    bf16, yt[p, k*CAP+j] = y_g[j, k*128+p] (output, transposed)

Phase A (per i-tile it of 22): h1T/h3T [128(i), CAP] = sum_k wT @ x tiles in
PSUM, then hT = silu(h1)*h3 into SBUF. Phase B (transposed): yT[d-tile] [128,
CAP] accumulated over the 22 i-tiles in PSUM (8 banks, one per d-tile), with
the w2 128x128 tile stationary and hT moving.

DMA model (measured): the 16 HW descriptor queues are SHARED by all engine
rings and serve ~350 GB/s aggregate in rough enqueue order; descriptors are
one per partition row (~130ns each, nearly size-independent), so 4KB+ rows
are required to reach full rate. Priority = enqueue order, so the whole
weight stream rides ONE compute-free ring (sync) in exact consumption order
(x, w13 halves 0..21, w2 pairs in phase-B order); delivery (~2.9us/pair)
then always beats PE consumption (~3.8us/pair) and phase A runs PE-bound
end-to-end. scalar carries only the x trigger so silus never queue behind a
backpressure-blocked trigger. Warmup matmuls accumulate zeros into the first
h1 PSUM bank (releases the HAM clock-gate during the ~13us DMA lead-in
without a dedicated bank or DCE-guard output). Exec time beyond the matmul
stream is fixed overhead: ~5.8us program start (excluded from the metric),
~2.3us final-bank cast+trigger+descriptor chain, ~8.8us end-of-NEFF epilogue.
"""

import numpy as np
from contextlib import ExitStack

import concourse.bass as bass
import concourse.bacc as bacc
import concourse.mybir as mybir
import concourse.tile as tile
from concourse.bass_utils import run_bass_kernel_spmd

E, I, D = 8, 2816, 1024
N_CORES = 8
NI, ND = I // 128, D // 128  # 22, 8

# storage dtype for weights/activations on-device: "bfloat16" (half HBM
# traffic, full PE rate) or "float32" (matmuls run as float32r, 2 cyc/row)
DT_NAME = "bfloat16"

_PROG_CACHE: dict = {}


def _build_program(cap: int, dt_name: str):
    DT = mybir.dt.float32r if dt_name == "float32" else getattr(mybir.dt, dt_name)
    f32 = mybir.dt.float32
    bf16 = mybir.dt.bfloat16
    NP = NI // 2  # w13/w2 DMAs batched as i-tile pairs
    nc = bacc.Bacc("TRN2", target_bir_lowering=False, debug=False)
    xt = nc.dram_tensor("xt", [128, ND * cap], DT, kind="ExternalInput").ap()
    w13 = nc.dram_tensor("w13", [NP, 128, 4 * D], DT, kind="ExternalInput").ap()
    w2s = nc.dram_tensor("w2s", [NP, 128, 2 * D], DT, kind="ExternalInput").ap()
    yt = nc.dram_tensor("yt", [128, ND * cap], bf16, kind="ExternalOutput").ap()

    with tile.TileContext(nc) as tc, ExitStack() as ctx:
        warmp = ctx.enter_context(tc.tile_pool(name="warm", bufs=1))
        xp = ctx.enter_context(tc.tile_pool(name="x", bufs=1))
        # w13p bufs=4: streaming pool — the sync-ring trigger for pair j
        # blocks until pair j-4's matmuls retire, which still keeps
        # delivery ~10us ahead of consumption. Fewer tile buffers also
        # shorten the end-of-program per-buffer semaphore check chain
        # (~115ns each) that the exec-time metric includes.
        w13p = ctx.enter_context(tc.tile_pool(name="w13", bufs=4))
        hp = ctx.enter_context(tc.tile_pool(name="h", bufs=NI))
        silp = ctx.enter_context(tc.tile_pool(name="sil", bufs=2))
        w2p = ctx.enter_context(tc.tile_pool(name="w2", bufs=1))
        yp = ctx.enter_context(tc.tile_pool(name="y", bufs=1))

        # DMA schedule (v3): the 16 HW descriptor queues are SHARED by all
        # engine rings and service descriptors in rough enqueue order at
        # ~350 GB/s aggregate; concurrently-enqueued transfers dilute each
        # other (a transfer completes only when its last descriptor drains
        # behind everything enqueued alongside it). So priority = enqueue
        # order, and the whole weight stream goes on ONE ring (sync, which
        # has no compute duties) in exact consumption order: x_h1, w13
        # halves s=0..21, then w2 halves in phase-B it-order. Delivery
        # (~2.9us/pair) then always beats PE consumption (~3.8us/pair).
        # scalar carries only x_h0 so its silus are never stuck behind a
        # backpressure-blocked DMA trigger; gpsimd is unused.
        xsb = xp.tile([128, ND * cap], DT)
        w13ts = [
            w13p.tile([128, 4 * D], DT, tag="w13", name=f"w13_{j}")
            for j in range(NP)
        ]
        w2ts = [
            w2p.tile([128, 2 * D], DT, tag=f"w2_{j}", name=f"w2_{j}")
            for j in range(NP)
        ]

        def w13_half(eng, s):
            j, h = s // 2, s % 2
            eng.dma_start(
                w13ts[j][:, h * 2 * D : (h + 1) * 2 * D],
                w13[j][:, h * 2 * D : (h + 1) * 2 * D],
            )

        # Descriptors are one-per-partition-row (~130ns service each,
        # roughly size-independent), so larger rows = more bandwidth:
        # x goes as ONE transfer (4KB+ rows) and w2 as pair transfers
        # (4KB rows) rather than halves (2KB rows).
        nc.scalar.dma_start(xsb[:], xt[:])
        # half 0 split into its w1 and w3 column blocks: the first 8
        # matmuls need only x + the w1 block, so their DMA gate drops by
        # 128 descriptors (~0.8us); 2KB rows are fine for just this pair
        nc.sync.dma_start(w13ts[0][:, 0:D], w13[0][:, 0:D])
        nc.sync.dma_start(w13ts[0][:, D : 2 * D], w13[0][:, D : 2 * D])
        for s in range(1, 2 * NP):
            w13_half(nc.sync, s)
        for j in range(NP):
            nc.sync.dma_start(w2ts[j][:], w2s[j])

        # PE warmup: matmuls on a zeroed tile release the HAM clock-gate
        # and cover the window until x + the first w13 half land (~13us).
        # They accumulate zeros directly INTO the first h1 PSUM bank, so no
        # separate warm bank (all 8 banks stay available: 6 hps + 2 free
        # for phase B's first banks) and no DCE-guard output is needed.
        hts = []
        with tc.tile_pool(name="hps", bufs=3, space="PSUM") as hps:
            wtile = warmp.tile([128, 640], DT)
            nc.vector.memset(wtile[:], 0.0)
            h1_first = hps.tile([128, cap], f32, tag="h1", name="h1")
            # data (x + first w13 half) lands at 12.8-16.1us across runs;
            # warm-end ~13.2us covers the common late-arrival cases so the
            # PE doesn't idle into a HAM clock demotion before phase A
            n_warm = 32
            for i in range(n_warm):
                nc.tensor.matmul(
                    h1_first[:],
                    wtile[:, 0:128],
                    wtile[:, 128 : 128 + cap],
                    start=(i == 0),
                    stop=False,
                )

            for j in range(NP):
                wt = w13ts[j]
                for half in range(2):
                    base = half * 2 * D
                    first = j == 0 and half == 0
                    if first:
                        h1 = h1_first
                    else:
                        h1 = hps.tile([128, cap], f32, tag="h1", name="h1")
                    h3 = hps.tile([128, cap], f32, tag="h3", name="h3")
                    for k in range(ND):
                        nc.tensor.matmul(
                            h1[:],
                            wt[:, base + k * 128 : base + (k + 1) * 128],
                            xsb[:, k * cap : (k + 1) * cap],
                            start=(k == 0 and not first),
                            stop=(k == ND - 1),
                        )
                    for k in range(ND):
                        nc.tensor.matmul(
                            h3[:],
                            wt[:, base + D + k * 128 : base + D + (k + 1) * 128],
                            xsb[:, k * cap : (k + 1) * cap],
                            start=(k == 0),
                            stop=(k == ND - 1),
                        )
                    sil = silp.tile([128, cap], f32)
                    nc.scalar.activation(
                        sil[:], h1[:], mybir.ActivationFunctionType.Silu
                    )
                    ht = hp.tile([128, cap], DT)
                    nc.vector.tensor_mul(ht[:], sil[:], h3[:])
                    hts.append(ht)

        # Phase B: yT[d-tile][128, cap] += w2tile.T @ hT (w2 stationary).
        # it-major prefix consumes w2 i-tiles in streaming-arrival order
        # (w2 is still arriving when B starts); k-major tail staggers bank
        # completion so PSUM drains overlap the remaining matmuls.
        # ITSPLIT=11 staggers bank completions every NI-ITSPLIT i-tiles
        # (~1.24us) — just above the ~1.04us descriptor-service time of one
        # 67KB drain, so the drain stream keeps pace with bank completion
        # and the last drain starts right after the last matmul.
        ITSPLIT = 11
        with tc.tile_pool(name="yps", bufs=1, space="PSUM") as yps:
            ypt = [
                yps.tile([128, cap], f32, tag=f"yps_{k}", name=f"yps_{k}")
                for k in range(ND)
            ]
            ysb = yp.tile([128, ND * cap], bf16)
            for it in range(ITSPLIT):
                for k in range(ND):
                    nc.tensor.matmul(
                        ypt[k][:],
                        w2ts[it // 2][
                            :, (it % 2) * D + k * 128 : (it % 2) * D + (k + 1) * 128
                        ],
                        hts[it][:],
                        start=(it == 0),
                        stop=False,
                    )
            for k in range(ND):
                for it in range(ITSPLIT, NI):
                    nc.tensor.matmul(
                        ypt[k][:],
                        w2ts[it // 2][
                            :, (it % 2) * D + k * 128 : (it % 2) * D + (k + 1) * 128
                        ],
                        hts[it][:],
                        start=False,
                        stop=(it == NI - 1),
                    )
                dst = ysb[:, k * cap : (k + 1) * cap]
                nc.vector.tensor_copy(dst, ypt[k][:])
                if k < ND - 2:
                    # one transfer per bank (128 row-descriptors); gpsimd's
                    # software-DMA path is slow, keep drains on sync/scalar
                    eng = nc.sync if k % 2 == 0 else nc.scalar
                    eng.dma_start(yt[:, k * cap : (k + 1) * cap], dst)
                else:
                    # last two banks: split along PARTITIONS (same total
                    # descriptor count) so the two ~600ns triggers run on
                    # both rings in parallel and each queue serves only 4
                    # rows — halves the post-matmul drain latency
                    cols = slice(k * cap, (k + 1) * cap)
                    nc.sync.dma_start(yt[0:64, cols], dst[0:64, :])
                    nc.scalar.dma_start(yt[64:128, cols], dst[64:128, :])

    nc.compile()
    return nc


def _get_program(cap: int, dt_name: str):
    key = (cap, dt_name)
    if key not in _PROG_CACHE:
        _PROG_CACHE[key] = _build_program(cap, dt_name)
    return _PROG_CACHE[key]


def _np_dt(dt_name: str):
    if dt_name == "float32":
        return np.float32
    import ml_dtypes

    return ml_dtypes.bfloat16


def _prep_weights(w1, w3, w2, dt_name):
    """Per-expert pretransposed/tiled weight arrays (see module docstring)."""
    npdt = _np_dt(dt_name)
    w13_all, w2s_all = [], []
    for e in range(E):
        # [I, D] -> [it, c, k, p] -> [it, p, k, c] -> [it, 128, 1024]
        a1 = w1[e].reshape(NI, 128, ND, 128).transpose(0, 3, 2, 1).reshape(NI, 128, D)
        a3 = w3[e].reshape(NI, 128, ND, 128).transpose(0, 3, 2, 1).reshape(NI, 128, D)
        # pairs of i-tiles: [11, 128, 4096] = [w1|w3] for it=2j then it=2j+1
        a13 = np.concatenate([a1, a3], axis=2).reshape(NI // 2, 2, 128, 2 * D)
        w13_all.append(
            np.ascontiguousarray(a13.transpose(0, 2, 1, 3)).reshape(
                NI // 2, 128, 4 * D
            ).astype(npdt)
        )
        # w2[e] [D, I] -> T [I, D] -> [22, 128, 1024] -> pairs [11, 128, 2048]
        a2 = w2[e].T.reshape(NI // 2, 2, 128, D)
        w2s_all.append(
            np.ascontiguousarray(a2.transpose(0, 2, 1, 3)).reshape(
                NI // 2, 128, 2 * D
            ).astype(npdt)
        )
    return w13_all, w2s_all


def kernel(x, w1, w2, w3, expert_indices, _trace=False):
    x = np.asarray(x, dtype=np.float32)
    w1 = np.asarray(w1, dtype=np.float32)
    w2 = np.asarray(w2, dtype=np.float32)
    w3 = np.asarray(w3, dtype=np.float32)
    idx = np.asarray(expert_indices).astype(np.int64)
    T, A = idx.shape
    npdt = _np_dt(DT_NAME)

    # Dedup: a token whose two slots pick the SAME expert is computed once
    # on that expert's core and its row written to both output slots.
    tok_lists = [np.nonzero((idx == e).any(axis=1))[0] for e in range(E)]
    counts = np.array([len(t) for t in tok_lists], dtype=np.int64)

    w13_all, w2s_all = _prep_weights(w1, w3, w2, DT_NAME)

    out = np.empty((T * A, D), dtype=np.float32)
    remaining = counts.copy()
    done = np.zeros(E, dtype=np.int64)
    last_res = None
    while remaining.max() > 0:
        cap = min(512, max(32, int(-(-remaining.max() // 8)) * 8))
        nc = _get_program(cap, DT_NAME)
        in_maps = []
        core_tok = []  # per-core token ids handled this round
        for e in range(E):
            n = int(min(remaining[e], cap))
            toks = tok_lists[e][done[e] : done[e] + n]
            core_tok.append(toks)
            xg = np.zeros((cap, D), dtype=np.float32)
            xg[:n] = x[toks]
            # [cap, D] -> T [D, cap] -> [k, 128, cap] -> [128, k, cap]
            xt_host = np.ascontiguousarray(
                xg.T.reshape(ND, 128, cap).transpose(1, 0, 2)
            ).reshape(128, ND * cap).astype(npdt)
            in_maps.append({"xt": xt_host, "w13": w13_all[e], "w2s": w2s_all[e]})
            remaining[e] -= n
            done[e] += n
        last_res = run_bass_kernel_spmd(
            nc, in_maps, core_ids=list(range(N_CORES)), trace=_trace
        )
        for e in range(E):
            toks = core_tok[e]
            if len(toks):
                # yt [128, 8*cap] -> [p, k, j] -> y[j, k*128+p]
                ye = (
                    last_res.results[e]["yt"]
                    .astype(np.float32)
                    .reshape(128, ND, cap)
                    .transpose(2, 1, 0)
                    .reshape(cap, D)
                )
                rr, aa = np.nonzero(idx[toks] == e)  # rows/slots to scatter
                out[toks[rr] * A + aa] = ye[rr]

    result = out.reshape(T, A, D)
    if _trace:
        return result, last_res
    return result

